# revision 4
# baseline (speedup 1.0000x reference)
"""PASA group-softmax downsample kernel for 8 Trainium2 NeuronCores. V4.

All-bf16, host parity planes [a][s][r][c], paired taps, software-pipelined
emission so PE never starves:

  pre: conv(0), conv(1), exp(0), den(0), recip(0)
  loop b: [conv(b+2) 1st half] rps(b) [conv(b+2) 2nd half] norm(b)
          exp(b+1) den(b+1) recip(b+1)
          bcast+mul+tree+out (b, ch=0..1)

Tap groups per (ch,block) unit: 3 kj-pairs (ki=0,1,2; kj in {0,1}),
1 ki-pair (kj=2; ki in {0,1}), 1 single (ki=2,kj=2).
Pair modes: AD = ACT copy psum->bf16 + DVE tensor_mul (2x bf16)
            AP = ACT copy + Pool mul, PD = Pool STT from PSUM,
            DD = DVE STT from PSUM (1x)
"""

import numpy as np
from contextlib import ExitStack

import ml_dtypes

import concourse.bass as bass
import concourse.bacc as bacc
import concourse.tile as tile
import concourse.mybir as mybir
import concourse.bass_utils as bass_utils
from concourse.ap import AP

F32 = mybir.dt.float32
F32R = mybir.dt.float32r
BF16 = mybir.dt.bfloat16
ALU = mybir.AluOpType
ACTF = mybir.ActivationFunctionType

N_CORES = 8
N, C, H, W = 4, 256, 128, 128
K = 3
GROUP = 8
CO = GROUP * K * K          # 72 conv output channels
OH, OW = H // 2, W // 2
HALF = OH // 2              # 32 output rows per core
BN_EPS = 1e-5

NBLK = 4
BROWS = HALF // NBLK        # 8 output rows per block
BPIX = BROWS * OW           # 512 pixels per block
PR = 33                     # plane rows
PC = 65                     # plane cols (odd plane uses 64, padded to 65)
XFREE = 2 * 2 * PR * PC     # [a][s][r][c] free elems per partition

# tap groups: (name, [(t, a, s, r0, c0) ...]) ; pair dim first entry order
# t = 3*ki + kj ; a = ki&1 ; r0 = ki>>1 ; s = kj&1 ; c0 = kj>>1
# P1..P3: kj-pairs (s is pair dim), P4: a-pair (kj=2), S: single
PAIRS = [
    ("kj01_ki0", [(0, 0, 0, 0, 0), (1, 0, 1, 0, 0)], "s"),
    ("kj01_ki1", [(3, 1, 0, 0, 0), (4, 1, 1, 0, 0)], "s"),
    ("kj01_ki2", [(6, 0, 0, 1, 0), (7, 0, 1, 1, 0)], "s"),
    ("kj2_ki01", [(2, 0, 0, 0, 1), (5, 1, 0, 0, 1)], "a"),
]
SINGLE = (8, 0, 0, 1, 1)    # ki=2, kj=2

# Per-unit config; unit u = 2*b + ch. pairs: 4 modes; single: 1 mode;
# adds: 8 engine chars for [c1 c2 c3 c4 a12 a34 aT aOut].
# Modes: AD = ACT copy + DVE mul (2x bf16), AP = ACT copy + Pool STT mul,
#        DD = DVE STT straight from PSUM. (Pool cannot read PSUM on HW.)
UNIT = dict(pairs=("AD", "DD", "AD", "DD"), single="AD", adds="dpddpddd")
CFG = dict(
    units=[dict(UNIT) for _ in range(8)],
    norm_eng="pool",
    mul4d=False,
    sigbp_bufs=2,   # pair PSUM tiles [128,1024] (2 banks each)
    sigbs_bufs=1,   # single PSUM tiles [128,512]
    conv_bufs=2,
    sbc_bufs=6,
    pk_bufs=6,
    acc_bufs=6,
    x_bufs=2,
)



def build_kernel(nc, cfg=CFG, timing_loop=True, emit_reps=1):
    xpl = nc.dram_tensor("xpl", [C, XFREE], BF16, kind="ExternalInput").ap()
    wconv = nc.dram_tensor("wconv", [128, 36 * CO], BF16, kind="ExternalInput").ap()
    sel = nc.dram_tensor("sel", [CO, 36 * 128], mybir.dt.float8e4, kind="ExternalInput").ap()
    bias = nc.dram_tensor("bias", [CO, 1], F32, kind="ExternalInput").ap()
    ones72 = nc.dram_tensor("ones72", [CO, CO], BF16, kind="ExternalInput").ap()
    nrep = nc.dram_tensor("nrep", [1, 1], mybir.dt.int32, kind="ExternalInput").ap()
    oh = nc.dram_tensor("oh", [C, HALF, OW], BF16, kind="ExternalOutput").ap()

    units = cfg["units"]

    with tile.TileContext(nc) as tc, ExitStack() as ctx:
        cpool = ctx.enter_context(tc.tile_pool(name="consts", bufs=1))
        xpool = ctx.enter_context(tc.tile_pool(name="xpl", bufs=cfg["x_bufs"]))
        spool = ctx.enter_context(tc.tile_pool(name="small", bufs=3))
        sbcool = ctx.enter_context(tc.tile_pool(name="sbc", bufs=cfg["sbc_bufs"]))
        pkpool = ctx.enter_context(tc.tile_pool(name="pk", bufs=cfg["pk_bufs"]))
        accpool = ctx.enter_context(tc.tile_pool(name="acc", bufs=cfg["acc_bufs"]))
        outpool = ctx.enter_context(tc.tile_pool(name="outs", bufs=2))
        convp = ctx.enter_context(tc.tile_pool(name="convp", bufs=cfg["conv_bufs"], space="PSUM"))
        sgbp = ctx.enter_context(tc.tile_pool(name="sgbp", bufs=cfg["sigbp_bufs"], space="PSUM"))
        sgbs = ctx.enter_context(tc.tile_pool(name="sgbs", bufs=cfg["sigbs_bufs"], space="PSUM"))
        denp = ctx.enter_context(tc.tile_pool(name="denp", bufs=1, space="PSUM"))

        wsb = cpool.tile([128, 36 * CO], BF16, tag="wsb")
        nc.sync.dma_start(wsb[:], wconv)
        selsb = cpool.tile([CO, 36 * 128], mybir.dt.float8e4, tag="selsb")
        nc.sync.dma_start(selsb[:], sel)
        bsb = cpool.tile([CO, 1], F32, tag="bsb")
        nc.sync.dma_start(bsb[:], bias)
        o72sb = cpool.tile([CO, CO], BF16, tag="o72sb")
        nc.sync.dma_start(o72sb[:], ones72)
        rsb_t = cpool.tile([1, 1], mybir.dt.int32, tag="rsb_t")
        nc.sync.dma_start(rsb_t[:], nrep)
        if timing_loop:
            with tc.tile_critical():
                nrep_v = nc.values_load(rsb_t[:], min_val=1, max_val=1 << 20,
                                        skip_runtime_bounds_check=True)
            loop_cm = tc.For_i(0, nrep_v, 1, hint_engines=(
                mybir.EngineType.PE, mybir.EngineType.DVE,
                mybir.EngineType.Activation, mybir.EngineType.Pool,
                mybir.EngineType.SP))
        else:
            import contextlib
            loop_cm = contextlib.nullcontext()
        with loop_cm:
          for _rep in range(emit_reps):
            xts = []
            for ch in range(2):
                xt = xpool.tile([128, XFREE], BF16, tag=f"x{ch}")
                nc.sync.dma_start(xt[:], xpl[128 * ch:128 * (ch + 1), :])
                xts.append(xt)

            def xv(ch):
                return xts[ch][:].rearrange(
                    "p (a s r c) -> p a s r c", a=2, s=2, c=PC)

            def tapview(ch, b, a, s, r0, c0):
                r = BROWS * b + r0
                return xv(ch)[:, a, s, r:r + BROWS, c0:c0 + OW]

            def pairview(ch, b, grp):
                _, taps, pdim = grp
                (t0, a0, s0, r00, c00), (t1, a1, s1, r01, c01) = taps
                r = BROWS * b + r00
                v = xv(ch)
                if pdim == "s":
                    assert a0 == a1 and r00 == r01 and c00 == c01
                    return v[:, a0, :, r:r + BROWS, c00:c00 + OW]
                else:
                    assert s0 == s1 and r00 == r01 and c00 == c01
                    return v[:, :, s0, r:r + BROWS, c00:c00 + OW]

            # per-block state carried across emission stages
            cps_t, sexp_t, rrow_t, sig_t = {}, {}, {}, {}

            def emit_conv(b, part):
                # part 0: taps 0..4 both ch ; part 1: taps 5..8 ; None: all
                if b not in cps_t:
                    cps = convp.tile([CO, BPIX], F32, tag="cps")
                    cps_t[b] = cps
                cps = cps_t[b]
                rng = {0: range(0, 5), 1: range(5, 9), None: range(9)}[part]
                for t in rng:
                    ki, kj = divmod(t, 3)
                    a, s, r0, c0 = ki & 1, kj & 1, ki >> 1, kj >> 1
                    for ch in range(2):
                        wv = wsb[:, (t * 2 + ch) * CO:(t * 2 + ch + 1) * CO]
                        nc.tensor.matmul(
                            cps[:].rearrange("p (h w) -> p h w", w=OW),
                            wv, tapview(ch, b, a, s, r0, c0),
                            start=(t == 0 and ch == 0),
                            stop=(t == 8 and ch == 1))

            def emit_expden(b):
                sexp = spool.tile([CO, BPIX], BF16, tag="sexp")
                nc.scalar.activation(sexp[:], cps_t[b][:], ACTF.Exp,
                                     bias=bsb[:], scale=1.0)
                sexp_t[b] = sexp
                dps = denp.tile([CO, BPIX], F32, tag="dps")
                nc.tensor.matmul(dps[:], o72sb[:], sexp[:], start=True, stop=True)
                rrow = spool.tile([CO, BPIX], F32R, tag="rrow")
                with nc.allow_low_precision(reason="f32r ~ f32 (12b mantissa)"):
                    nc.vector.reciprocal(rrow[:], dps[:])
                rrow_t[b] = rrow

            def emit_norm(b):
                sig = spool.tile([CO, BPIX], BF16, tag="sig")
                if cfg["norm_eng"] == "pool":
                    nc.gpsimd.tensor_mul(sig[:], rrow_t[b][:], sexp_t[b][:])
                else:
                    nc.vector.scalar_tensor_tensor(
                        sig[:], rrow_t[b][:], 1.0, sexp_t[b][:],
                        ALU.bypass, ALU.mult)
                sig_t[b] = sig

            def emit_bcast(b, ch):
                ucfg = units[2 * b + ch]
                sig = sig_t[b]
                adds = ucfg["adds"]

                def aeng(i):
                    return nc.vector if adds[i] == "d" else nc.gpsimd

                pks = []
                for gi, grp in enumerate(PAIRS):
                    mode = ucfg["pairs"][gi]
                    taps = grp[1]
                    sbp = sgbp.tile([128, 2 * BPIX], F32, tag="sbpp")
                    for half, (t, *_rest) in enumerate(taps):
                        sv = selsb[:, (t * 2 + ch) * 128:(t * 2 + ch + 1) * 128]
                        nc.tensor.matmul(
                            sbp[:, half * BPIX:(half + 1) * BPIX],
                            sv, sig[:], start=True, stop=True)
                    pk = pkpool.tile([128, 2 * BPIX], BF16, tag="pkp")
                    pv = pairview(ch, b, grp)
                    src_t = sbp
                    if mode in ("AD", "AP"):
                        sbc = sbcool.tile([128, 2 * BPIX], BF16, tag="sbcp")
                        nc.scalar.copy(sbc[:], sbp[:])
                        src_t = sbc
                    if mode == "AD" and cfg["mul4d"]:
                        nc.vector.tensor_mul(
                            pk[:].rearrange("p (s h w) -> p s h w", s=2, w=OW),
                            src_t[:].rearrange("p (s h w) -> p s h w",
                                               s=2, w=OW), pv)
                    else:
                        # 3D per-half ops (STT/TT must be <=3D for the BIR
                        # verifier; Pool additionally cannot read PSUM)
                        for half, (t, a, s, r0, c0) in enumerate(grp[1]):
                            o3 = pk[:, half * BPIX:(half + 1) * BPIX]                                 .rearrange("p (h w) -> p h w", w=OW)
                            i3 = src_t[:, half * BPIX:(half + 1) * BPIX]                                 .rearrange("p (h w) -> p h w", w=OW)
                            pvh = tapview(ch, b, a, s, r0, c0)
                            if mode == "AD":
                                nc.vector.tensor_mul(o3, i3, pvh)
                            elif mode == "AP":
                                nc.gpsimd.tensor_mul(o3, i3, pvh)
                            else:  # DD
                                nc.vector.scalar_tensor_tensor(
                                    o3, i3, 1.0, pvh, ALU.bypass, ALU.mult)
                    pks.append(pk)
                # single tap
                t, a, s, r0, c0 = SINGLE
                sbs = sgbs.tile([128, BPIX], F32, tag="sbps")
                sv = selsb[:, (t * 2 + ch) * 128:(t * 2 + ch + 1) * 128]
                nc.tensor.matmul(sbs[:], sv, sig[:], start=True, stop=True)
                pks_s = pkpool.tile([128, BPIX], BF16, tag="pks")
                pv = tapview(ch, b, a, s, r0, c0)
                smode = ucfg["single"]
                if smode in ("AD", "AP"):
                    sbc = sbcool.tile([128, BPIX], BF16, tag="sbcs")
                    nc.scalar.copy(sbc[:], sbs[:])
                    sb3 = sbc[:].rearrange("p (h w) -> p h w", w=OW)
                    pk3 = pks_s[:].rearrange("p (h w) -> p h w", w=OW)
                    if smode == "AD":
                        nc.vector.tensor_mul(pk3, sb3, pv)
                    else:
                        nc.gpsimd.tensor_mul(pk3, sb3, pv)
                else:
                    nc.vector.scalar_tensor_tensor(
                        pks_s[:].rearrange("p (h w) -> p h w", w=OW),
                        sbs[:].rearrange("p (h w) -> p h w", w=OW),
                        1.0, pv, ALU.bypass, ALU.mult)

                # tree: 4 pair-collapses, a12, a34, aT, aOut(+single)
                cs = []
                for i, pk in enumerate(pks):
                    c = accpool.tile([128, BPIX], BF16, tag="acc")
                    aeng(i).tensor_add(c[:], pk[:, 0:BPIX], pk[:, BPIX:2 * BPIX])
                    cs.append(c)
                c12 = accpool.tile([128, BPIX], BF16, tag="acc")
                aeng(4).tensor_add(c12[:], cs[0][:], cs[1][:])
                c34 = accpool.tile([128, BPIX], BF16, tag="acc")
                aeng(5).tensor_add(c34[:], cs[2][:], cs[3][:])
                call = accpool.tile([128, BPIX], BF16, tag="acc")
                aeng(6).tensor_add(call[:], c12[:], c34[:])
                outsb = outpool.tile([128, BPIX], BF16, tag="outsb")
                aeng(7).tensor_add(outsb[:], call[:], pks_s[:])
                nc.sync.dma_start(
                    oh[128 * ch:128 * (ch + 1), BROWS * b:BROWS * (b + 1), :],
                    outsb[:].rearrange("p (h w) -> p h w", w=OW))

            # ---- software-pipelined emission ----
            emit_conv(0, None)
            emit_conv(1, None)
            emit_expden(0)
            emit_norm(0)
            for b in range(NBLK):
                if b + 2 < NBLK:
                    emit_conv(b + 2, None)
                if b + 1 < NBLK:
                    emit_expden(b + 1)
                    emit_norm(b + 1)
                emit_bcast(b, 0)
                emit_bcast(b, 1)
    nc.compile()
    return nc


def prepare_const_inputs(conv_w, bn_gamma, bn_beta, bn_mean, bn_var):
    inv = 1.0 / np.sqrt(bn_var.astype(np.float64) + BN_EPS)
    scale = (bn_gamma.astype(np.float64) * inv)
    wp = conv_w.astype(np.float64) * scale[:, None, None, None]
    bias = (bn_beta.astype(np.float64)
            - bn_mean.astype(np.float64) * scale).astype(np.float32)

    wconv = np.zeros((128, 36, CO), np.float32)
    selm = np.zeros((CO, 36, 128), np.float32)
    for t in range(9):
        ki, kj = divmod(t, 3)
        for ch in range(2):
            blk = t * 2 + ch
            wconv[:, blk, :] = wp[:, 128 * ch:128 * (ch + 1), ki, kj].T
            for g4 in range(4):
                r = (ch * 4 + g4) * 9 + t
                selm[r, blk, 32 * g4:32 * (g4 + 1)] = 1.0
    return {
        "wconv": np.ascontiguousarray(
            wconv.reshape(128, 36 * CO)).astype(ml_dtypes.bfloat16),
        "sel": np.ascontiguousarray(
            selm.reshape(CO, 36 * 128)).astype(ml_dtypes.float8_e4m3),
        "bias": bias.reshape(CO, 1),
        "ones72": np.ones((CO, CO), ml_dtypes.bfloat16),
    }


def prepare_x_core(x, core):
    n, half = divmod(core, 2)
    if half == 0:
        rows = np.concatenate([x[n, :, 1:2, :], x[n, :, 0:2 * HALF, :]], axis=1)
    else:
        rows = x[n, :, 2 * HALF - 1:H, :]
    padded = np.concatenate([rows[:, :, 1:2], rows], axis=2)  # [C, 65, 129]
    xpl = np.zeros((C, 2, 2, PR, PC), np.float32)
    for a in range(2):
        sub = padded[:, a::2, :]                    # [C, 33 or 32, 129]
        nr = sub.shape[1]
        xpl[:, a, 0, :nr, :] = sub[:, :, 0::2]      # even cols (65)
        xpl[:, a, 1, :nr, :64] = sub[:, :, 1::2]    # odd cols (64)
    return np.ascontiguousarray(
        xpl.reshape(C, XFREE)).astype(ml_dtypes.bfloat16)


_CACHE = {}


def _get_nc(key="v10", cfg=None, timing_loop=True):
    if key not in _CACHE:
        nc = bacc.Bacc("TRN2", target_bir_lowering=False, debug=False,
                       num_devices=N_CORES)
        _CACHE[key] = build_kernel(nc, cfg=cfg or CFG, timing_loop=timing_loop)
    return _CACHE[key]


def run_on_cores(inputs, reps=1):
    nc = _get_nc()
    consts = prepare_const_inputs(
        inputs["conv_w"], inputs["bn_gamma"], inputs["bn_beta"],
        inputs["bn_mean"], inputs["bn_var"])
    consts["nrep"] = np.array([[reps]], np.int32)
    x = np.asarray(inputs["x"])
    in_maps = []
    for core in range(N_CORES):
        m = dict(consts)
        m["xpl"] = prepare_x_core(x, core)
        in_maps.append(m)
    res = bass_utils.run_bass_kernel_spmd(nc, in_maps, core_ids=list(range(N_CORES)))
    out = np.empty((N, C, OH, OW), np.float32)
    for core in range(N_CORES):
        n, half = divmod(core, 2)
        out[n, :, HALF * half:HALF * (half + 1), :] = \
            res.results[core]["oh"].astype(np.float32)
    return out


def kernel(**inputs):
    return run_on_cores(inputs, reps=1)
